# revision 1
# baseline (speedup 1.0000x reference)
"""Trainium2 Bass kernel for nn_ClusterNet (3-layer linear GraphSAGE + max-pool + log_softmax).

Strategy
--------
The network is linear up to the final log_softmax:
    h3 = sum_{k=0..3} (M^k xt) Ct_k,   xt = [x | 1]  (bias column rides along),
where M = D^-1 A is the mean-aggregation operator and Ct_k are host-folded
products of the small weight matrices (weights are replicated; folding them is
constant preprocessing).

Device work (8 NeuronCores, graph-sharded by batch-id ranges so aggregation
output rows and pooling are core-local):
  - 3 rounds of "apply M": gather neighbor rows via nc.gpsimd.dma_gather from
    f32 tables (int16 indices -> 4 table windows), degree-bucketed so a
    strided tree-add reduces each node's K slots, then a recip scale.
  - window partials are recombined by a tiny add/scale launch; the host only
    PERMUTES rows between launches (no arithmetic on tensor data).
  - tail launch: h3 = sum_k y_k @ Ct_k via PE (transpose + accumulating
    matmuls), per-graph max-pool via a local dma_gather + tree-max, logits and
    log_softmax on-device. Output [8 graphs, 8] per core, assembled on host.
"""
import os
import sys

sys.path.insert(0, '/opt/trn_rl_repo')

import numpy as np

import concourse.bass as bass
import concourse.bacc as bacc
import concourse.tile as tile
import concourse.mybir as mybir
from concourse import bass_utils
from concourse.masks import make_identity

NCORES = 8
F = 64            # table row width (f32, 256B rows; cols 0..32 used)
FIN = 33          # x(32) + ones column
NW = 4            # table windows (int16 index limit)
BUCKETS = [1, 2, 3, 4, 6, 8, 12, 16, 24, 32, 48, 64]
CALL_MAX = 8192   # max indices per dma_gather call
TRACE = bool(os.environ.get("KERNEL_TRACE"))

LAST_EXEC_NS = []   # per-launch exec_time_ns when KERNEL_TRACE=1

if TRACE:
    # NTFF profiling shim: the image's antenv lacks axon_hooks, so register it
    # ourselves (dev-only; graders run with KERNEL_TRACE unset).
    import types

    if "antenv.axon_hooks" not in sys.modules:
        _m = types.ModuleType("antenv.axon_hooks")
        _m._hook = None
        _m.set_axon_ntff_profile_hook = lambda h: setattr(_m, "_hook", h)
        _m.get_axon_ntff_profile_hook = lambda: _m._hook
        sys.modules["antenv.axon_hooks"] = _m
        try:
            from trn_agent_boot.trn_boot import _ntff_profile_via_ctypes
            _m._hook = _ntff_profile_via_ctypes("/opt/axon/libaxon_pjrt.so")
        except Exception:
            _m._hook = None
    bass_utils.upload_artifacts = lambda tmpdir: f"local:{tmpdir}"

_prog_cache = {}


def _roundup(a, b):
    return (a + b - 1) // b * b


# ----------------------------------------------------------------- host plan
class Plan:
    pass


def build_plan(edge_index, batch, N, G):
    pl = Plan()
    src = np.asarray(edge_index[0], np.int64)
    dst = np.asarray(edge_index[1], np.int64)
    batch = np.asarray(batch, np.int64)
    E = src.shape[0]
    gpc = G // NCORES  # graphs per core

    core_of_graph = np.arange(G) // gpc
    node_core = core_of_graph[batch]                      # [N]
    n0 = np.searchsorted(batch, np.arange(0, G, gpc))     # core node range start
    n1 = np.append(n0[1:], N)
    NL = (n1 - n0).astype(np.int64)
    NLpad = int(_roundup(NL.max(), 128))
    pl.n0, pl.n1, pl.NL, pl.NLpad, pl.gpc = n0, n1, NL, NLpad, gpc

    deg = np.bincount(dst, minlength=N)
    recip = np.where(deg > 0, 1.0 / np.maximum(deg, 1), 0.0).astype(np.float32)

    TL = NCORES * NLpad
    WROW = _roundup((TL + NW - 1) // NW, 128)
    assert WROW + 1 <= 32767, WROW
    pl.WROW = WROW

    # logical table row of node n
    tl = node_core[np.arange(N)] * NLpad + (np.arange(N) - n0[node_core])
    pl.tl = tl

    src_w = tl[src] // WROW          # window of each edge's source
    src_li = tl[src] - src_w * WROW  # local row within window

    # per-core local edges
    dst_core = node_core[dst]
    ecore = [np.nonzero(dst_core == c)[0] for c in range(NCORES)]

    # per core, per window: CSR of edges grouped by (window, local dst)
    # and per-window degree
    pl.win_nodes = []   # [c][w] -> local node ids with deg_w>0, bucket-sorted
    pl.win_bucket = []  # [c][w] -> bucket K per such node
    pl.win_adj = []     # [c][w] -> dict arrays: concatenated slot indices per node
    nodes_wb = np.zeros((NCORES, NW, len(BUCKETS)), np.int64)
    for c in range(NCORES):
        e = ecore[c]
        ld = dst[e] - n0[c]
        w = src_w[e]
        li = src_li[e]
        pl.win_nodes.append([])
        pl.win_bucket.append([])
        pl.win_adj.append([])
        for wi in range(NW):
            m = w == wi
            ldw, liw = ld[m], li[m]
            order = np.argsort(ldw, kind='stable')
            ldw, liw = ldw[order], liw[order]
            uniq, counts = np.unique(ldw, return_counts=True)
            bidx = np.searchsorted(BUCKETS, counts)
            assert bidx.max(initial=0) < len(BUCKETS)
            # sort nodes by bucket
            no = np.argsort(bidx, kind='stable')
            pl.win_nodes[c].append(uniq[no])
            pl.win_bucket[c].append(bidx[no])
            starts = np.concatenate([[0], np.cumsum(counts)])
            pl.win_adj[c].append((uniq, starts, liw))
            for b in range(len(BUCKETS)):
                nodes_wb[c, wi, b] = int((bidx == b).sum())

    # global (max-over-core) group counts per (window, bucket)
    groups_wb = np.zeros((NW, len(BUCKETS)), np.int64)
    for wi in range(NW):
        for b in range(len(BUCKETS)):
            groups_wb[wi, b] = (int(nodes_wb[:, wi, b].max()) + 127) // 128
    pl.groups_wb = groups_wb

    # static call list: (window, K, G, sbase, idx_col_offset, num_idxs)
    calls = []
    sbase = 0
    icol = 0
    for wi in range(NW):
        for b, K in enumerate(BUCKETS):
            g_left = int(groups_wb[wi, b])
            while g_left > 0:
                Gc = min(g_left, CALL_MAX // (128 * K), 64)
                if Gc == 0:
                    Gc = 1
                ni = Gc * K * 128
                calls.append(dict(w=wi, K=K, G=Gc, b=b, sbase=sbase,
                                  icol=icol, ni=ni))
                sbase += Gc * 128
                icol += ni // 16
                g_left -= Gc
    pl.calls = calls
    pl.STOT = sbase
    pl.IDXC = icol

    # per-core idx buffer + recip buffer + S-row -> local-node map
    pl.idxbuf = np.full((NCORES, 128, pl.IDXC), WROW, np.int16)
    pl.recbuf = np.zeros((NCORES, 128, pl.STOT // 128), np.float32)
    pl.srow_node = np.full((NCORES, pl.STOT), -1, np.int64)  # local node or -1
    for c in range(NCORES):
        consumed = np.zeros((NW, len(BUCKETS)), np.int64)
        for call in calls:
            wi, K, Gc, b = call['w'], call['K'], call['G'], call['b']
            nodes = pl.win_nodes[c][wi]
            bks = pl.win_bucket[c][wi]
            uniq, starts, liw = pl.win_adj[c][wi]
            b_lo = int(np.searchsorted(bks, b))
            b_hi = int(np.searchsorted(bks, b + 1))
            cur = b_lo + int(consumed[wi, b])
            r = min(b_hi - cur, Gc * 128)
            L = np.full((Gc * K * 128,), pl.WROW, np.int16)  # default zero row
            if r > 0:
                nb = nodes[cur:cur + r]
                t = np.arange(r)
                pl.srow_node[c, call['sbase'] + t] = nb
                ui = np.searchsorted(uniq, nb)
                d = (starts[ui + 1] - starts[ui]).astype(np.int64)
                tot = int(d.sum())
                tt = np.repeat(t, d)
                off = np.concatenate([[0], np.cumsum(d)[:-1]])
                jj = np.arange(tot) - np.repeat(off, d)
                L[((tt // 128) * K + jj) * 128 + (tt % 128)] = \
                    liw[np.repeat(starts[ui], d) + jj]
                consumed[wi, b] += r
            ni = call['ni']
            Lw = L.reshape(ni // 16, 16).T  # [16, ni/16]
            pl.idxbuf[c, :, call['icol']:call['icol'] + ni // 16] = \
                np.tile(Lw, (8, 1))
    # real recip values
    for c in range(NCORES):
        rows = np.nonzero(pl.srow_node[c] >= 0)[0]
        gnodes = pl.srow_node[c, rows] + pl.n0[c]
        pl.recbuf[c, rows % 128, rows // 128] = recip[gnodes]
    return pl


def plan_from_inputs(edge_index, batch):
    return build_plan(edge_index, batch, batch.shape[0], int(batch.max()) + 1)


# ----------------------------------------------------------- device programs
def prog_agg(pl):
    key = ('agg', pl.STOT, pl.IDXC, len(pl.calls), pl.WROW)
    if key in _prog_cache:
        return _prog_cache[key]
    nc = bacc.Bacc("TRN2", target_bir_lowering=False, debug=False,
                   num_devices=NCORES)
    tabs = [nc.dram_tensor(f"tab{w}", (pl.WROW + 1, F), mybir.dt.float32,
                           kind="ExternalInput").ap() for w in range(NW)]
    idx = nc.dram_tensor("idx", (128, pl.IDXC), mybir.dt.int16,
                         kind="ExternalInput").ap()
    rec = nc.dram_tensor("rec", (128, pl.STOT // 128), mybir.dt.float32,
                         kind="ExternalInput").ap()
    S = nc.dram_tensor("S", (pl.STOT, F), mybir.dt.float32,
                       kind="ExternalOutput").ap()

    with tile.TileContext(nc) as tc:
        with tc.tile_pool(name="io", bufs=1) as iop, \
             tc.tile_pool(name="g", bufs=4) as gp, \
             tc.tile_pool(name="st", bufs=4) as stp:
            idx_t = iop.tile([128, pl.IDXC], mybir.dt.int16)
            nc.sync.dma_start(out=idx_t[:], in_=idx[:, :])
            rec_t = iop.tile([128, pl.STOT // 128], mybir.dt.float32)
            nc.sync.dma_start(out=rec_t[:], in_=rec[:, :])
            for call in pl.calls:
                wi, K, Gc, ni = call['w'], call['K'], call['G'], call['ni']
                icol, sbase = call['icol'], call['sbase']
                t = gp.tile([128, Gc * K * F], mybir.dt.float32, tag="g")
                nc.gpsimd.dma_gather(
                    out_ap=t[:].rearrange("p (b f) -> p b f", f=F),
                    in_ap=tabs[wi][:],
                    idxs_ap=idx_t[:, icol:icol + ni // 16],
                    num_idxs=ni, num_idxs_reg=ni, elem_size=F,
                    single_packet=False)
                tv = t[:].rearrange("p (g k f) -> p g k f", g=Gc, k=K)
                kk = K
                while kk > 1:
                    h = kk // 2
                    nc.vector.tensor_add(
                        out=tv[:, :, :h, :],
                        in0=tv[:, :, :h, :],
                        in1=tv[:, :, h:2 * h, :])
                    if kk % 2 == 1:
                        nc.vector.tensor_add(
                            out=tv[:, :, 0, :],
                            in0=tv[:, :, 0, :],
                            in1=tv[:, :, kk - 1, :])
                    kk = h
                stg = stp.tile([128, Gc * F], mybir.dt.float32, tag="st")
                rbc = rec_t[:, sbase // 128:sbase // 128 + Gc]
                nc.vector.tensor_mul(
                    out=stg[:].rearrange("p (g f) -> p g f", f=F),
                    in0=tv[:, :, 0, :],
                    in1=rbc.unsqueeze(2).broadcast_to([128, Gc, F]))
                nc.sync.dma_start(
                    out=S[sbase:sbase + Gc * 128, :].rearrange(
                        "(g p) f -> p g f", p=128),
                    in_=stg[:].rearrange("p (g f) -> p g f", f=F))
    nc.compile()
    _prog_cache[key] = nc
    return nc


def prog_comb(pl):
    key = ('comb', pl.NLpad)
    if key in _prog_cache:
        return _prog_cache[key]
    nc = bacc.Bacc("TRN2", target_bir_lowering=False, debug=False,
                   num_devices=NCORES)
    NB = pl.NLpad // 128
    Ss = [nc.dram_tensor(f"S{w}", (pl.NLpad, F), mybir.dt.float32,
                         kind="ExternalInput").ap() for w in range(NW)]
    y = nc.dram_tensor("y", (pl.NLpad, F), mybir.dt.float32,
                       kind="ExternalOutput").ap()
    with tile.TileContext(nc) as tc:
        with tc.tile_pool(name="p", bufs=1) as pp:
            ts = []
            for w in range(NW):
                t = pp.tile([128, NB * F], mybir.dt.float32, tag=f"s{w}")
                nc.sync.dma_start(
                    out=t[:].rearrange("p (b f) -> p b f", f=F),
                    in_=Ss[w][:, :].rearrange("(b p) f -> p b f", p=128))
                ts.append(t)
            nc.vector.tensor_add(out=ts[0][:], in0=ts[0][:], in1=ts[1][:])
            nc.vector.tensor_add(out=ts[2][:], in0=ts[2][:], in1=ts[3][:])
            nc.vector.tensor_add(out=ts[0][:], in0=ts[0][:], in1=ts[2][:])
            nc.sync.dma_start(
                out=y[:, :].rearrange("(b p) f -> p b f", p=128),
                in_=ts[0][:].rearrange("p (b f) -> p b f", f=F))
    nc.compile()
    _prog_cache[key] = nc
    return nc


def prog_tail(pl, KG):
    key = ('tail', pl.NLpad, KG)
    if key in _prog_cache:
        return _prog_cache[key]
    nc = bacc.Bacc("TRN2", target_bir_lowering=False, debug=False,
                   num_devices=NCORES)
    NB = pl.NLpad // 128
    Ss = [nc.dram_tensor(f"S{w}", (pl.NLpad, F), mybir.dt.float32,
                         kind="ExternalInput").ap() for w in range(NW)]
    ys = [nc.dram_tensor(f"y{k}", (pl.NLpad, F), mybir.dt.float32,
                         kind="ExternalInput").ap() for k in range(3)]
    C = nc.dram_tensor("C", (4 * F, F), mybir.dt.float32,
                       kind="ExternalInput").ap()
    Wo = nc.dram_tensor("Wo", (F, 8), mybir.dt.float32,
                        kind="ExternalInput").ap()
    bo = nc.dram_tensor("bo", (8, 8), mybir.dt.float32,
                        kind="ExternalInput").ap()
    ninf = nc.dram_tensor("ninf", (128, F), mybir.dt.float32,
                          kind="ExternalInput").ap()
    pidx = nc.dram_tensor("pidx", (128, KG * 8 // 16), mybir.dt.int16,
                          kind="ExternalInput").ap()
    h3tab = nc.dram_tensor("h3tab", (pl.NLpad + 128, F), mybir.dt.float32,
                           kind="Internal").ap()
    out = nc.dram_tensor("out", (8, 8), mybir.dt.float32,
                         kind="ExternalOutput").ap()

    B = KG * 8 // 128  # pooling free blocks
    with tile.TileContext(nc) as tc:
        with tc.tile_pool(name="big", bufs=1) as bigp, \
             tc.tile_pool(name="wk", bufs=3) as wk, \
             tc.tile_pool(name="ps", bufs=2, space="PSUM") as ps:
            Ct = bigp.tile([128, 2 * F], mybir.dt.float32)  # [[C0;C1],[C2;C3]]
            nc.sync.dma_start(
                out=Ct[:].rearrange("p (k f) -> p k f", f=F),
                in_=C[:, :].rearrange("(k p) f -> p k f", p=128))
            ident = bigp.tile([128, 128], mybir.dt.float32)
            make_identity(nc, ident[:])
            Wot = bigp.tile([F, 8], mybir.dt.float32)
            nc.sync.dma_start(out=Wot[:], in_=Wo[:, :])
            bot = bigp.tile([8, 8], mybir.dt.float32)
            nc.sync.dma_start(out=bot[:], in_=bo[:, :])
            ninft = bigp.tile([128, F], mybir.dt.float32)
            nc.sync.dma_start(out=ninft[:], in_=ninf[:, :])
            nc.sync.dma_start(out=h3tab[pl.NLpad:pl.NLpad + 128, :], in_=ninft[:])
            pidx_t = bigp.tile([128, KG * 8 // 16], mybir.dt.int16)
            nc.sync.dma_start(out=pidx_t[:], in_=pidx[:, :])

            # h3 blocks: pack [y0|y1] and [y2|y3] pairs so one [128,128]
            # transpose + one matmul with stacked C rows handles two terms.
            CB = 25
            for c0 in range(0, NB, CB):
                cb = min(CB, NB - c0)
                rows = slice(c0 * 128, (c0 + cb) * 128)
                p01 = wk.tile([128, CB * 128], mybir.dt.float32, tag="p01")
                p23 = wk.tile([128, CB * 128], mybir.dt.float32, tag="p23")
                pv01 = p01[:].rearrange("p (b t) -> p b t", t=128)
                pv23 = p23[:].rearrange("p (b t) -> p b t", t=128)
                nc.sync.dma_start(
                    out=pv01[:, :cb, 0:F],
                    in_=ys[0][rows, :].rearrange("(b p) f -> p b f", p=128))
                nc.sync.dma_start(
                    out=pv01[:, :cb, F:128],
                    in_=ys[1][rows, :].rearrange("(b p) f -> p b f", p=128))
                nc.sync.dma_start(
                    out=pv23[:, :cb, 0:F],
                    in_=ys[2][rows, :].rearrange("(b p) f -> p b f", p=128))
                nc.sync.dma_start(
                    out=pv23[:, :cb, F:128],
                    in_=Ss[0][rows, :].rearrange("(b p) f -> p b f", p=128))
                for w in range(1, NW):
                    t = wk.tile([128, CB * F], mybir.dt.float32, tag="sw")
                    nc.sync.dma_start(
                        out=t[:, :cb * F].rearrange("p (b f) -> p b f", f=F),
                        in_=Ss[w][rows, :].rearrange("(b p) f -> p b f", p=128))
                    nc.vector.tensor_add(
                        out=pv23[:, :cb, F:128], in0=pv23[:, :cb, F:128],
                        in1=t[:, :cb * F].rearrange("p (b f) -> p b f", f=F))
                h3c = wk.tile([128, CB * F], mybir.dt.float32, tag="h3c")
                for blk in range(cb):
                    hp = ps.tile([128, F], mybir.dt.float32, space="PSUM",
                                 tag="hp")
                    for half, pv in ((0, pv01), (1, pv23)):
                        tp = ps.tile([128, 128], mybir.dt.float32,
                                     space="PSUM", tag="tp")
                        nc.tensor.transpose(out=tp[:], in_=pv[:, blk, :],
                                            identity=ident[:])
                        ykT = wk.tile([128, 128], mybir.dt.float32, tag="ykT")
                        nc.scalar.copy(out=ykT[:], in_=tp[:])
                        nc.tensor.matmul(
                            out=hp[:], lhsT=ykT[:],
                            rhs=Ct[:, half * F:(half + 1) * F],
                            start=(half == 0), stop=(half == 1))
                    nc.scalar.copy(out=h3c[:, blk * F:(blk + 1) * F], in_=hp[:])
                nc.sync.dma_start(
                    out=h3tab[rows, :].rearrange("(b p) f -> p b f", p=128),
                    in_=h3c[:, :cb * F].rearrange("p (b f) -> p b f", f=F))

            # pooling gather: slot i -> (p=i%128, b=i//128); p = g*16+chunk
            nslots = KG * 8
            pt = bigp.tile([128, B * F], mybir.dt.float32)
            half = nslots // 2
            nc.gpsimd.dma_gather(
                out_ap=pt[:, :half // 128 * F].rearrange("p (b f) -> p b f", f=F),
                in_ap=h3tab[:], idxs_ap=pidx_t[:, :half // 16],
                num_idxs=half, num_idxs_reg=half, elem_size=F,
                single_packet=False)
            nc.gpsimd.dma_gather(
                out_ap=pt[:, half // 128 * F:].rearrange("p (b f) -> p b f", f=F),
                in_ap=h3tab[:], idxs_ap=pidx_t[:, half // 16:],
                num_idxs=half, num_idxs_reg=half, elem_size=F,
                single_packet=False)
            # tree-max over B blocks
            pv = pt[:].rearrange("p (b f) -> p b f", f=F)
            bb = B
            while bb > 1:
                h = bb // 2
                nc.vector.tensor_tensor(
                    out=pv[:, :h, :], in0=pv[:, :h, :],
                    in1=pv[:, h:2 * h, :], op=mybir.AluOpType.max)
                if bb % 2 == 1:
                    nc.vector.tensor_tensor(
                        out=pv[:, 0, :], in0=pv[:, 0, :], in1=pv[:, bb - 1, :],
                        op=mybir.AluOpType.max)
                bb = h
            # transpose [128,F] -> [F,128]; then max over 16 chunks per graph
            tp2 = ps.tile([F, 128], mybir.dt.float32, space="PSUM", tag="tp")
            nc.tensor.transpose(out=tp2[:], in_=pv[:, 0, :], identity=ident[:])
            pooledC = wk.tile([F, 128], mybir.dt.float32, tag="pc")
            nc.scalar.copy(out=pooledC[:], in_=tp2[:])
            pcv = pooledC[:].rearrange("f (g c) -> f g c", c=16)
            cc = 16
            while cc > 1:
                h = cc // 2
                nc.vector.tensor_tensor(
                    out=pcv[:, :, :h], in0=pcv[:, :, :h],
                    in1=pcv[:, :, h:2 * h], op=mybir.AluOpType.max)
                cc = h
            pooledT = wk.tile([F, 8], mybir.dt.float32, tag="pt")
            nc.vector.tensor_copy(out=pooledT[:], in_=pcv[:, :, 0])
            # logits = pooled @ Wo + bo
            lg = ps.tile([8, 8], mybir.dt.float32, space="PSUM", tag="lg")
            nc.tensor.matmul(out=lg[:], lhsT=pooledT[:], rhs=Wot[:],
                             start=True, stop=True)
            lgs = wk.tile([8, 8], mybir.dt.float32, tag="lgs")
            nc.vector.tensor_add(out=lgs[:], in0=lg[:], in1=bot[:])
            # log_softmax along free dim
            mx = wk.tile([8, 1], mybir.dt.float32, tag="mx")
            nc.vector.tensor_reduce(out=mx[:], in_=lgs[:],
                                    axis=mybir.AxisListType.X,
                                    op=mybir.AluOpType.max)
            nc.vector.tensor_scalar(out=lgs[:], in0=lgs[:], scalar1=mx[:, :1],
                                    scalar2=None,
                                    op0=mybir.AluOpType.subtract)
            ex = wk.tile([8, 8], mybir.dt.float32, tag="ex")
            nc.scalar.activation(out=ex[:], in_=lgs[:],
                                 func=mybir.ActivationFunctionType.Exp)
            sm = wk.tile([8, 1], mybir.dt.float32, tag="sm")
            nc.vector.tensor_reduce(out=sm[:], in_=ex[:],
                                    axis=mybir.AxisListType.X,
                                    op=mybir.AluOpType.add)
            lns = wk.tile([8, 1], mybir.dt.float32, tag="lns")
            nc.scalar.activation(out=lns[:], in_=sm[:],
                                 func=mybir.ActivationFunctionType.Ln)
            nc.vector.tensor_scalar(out=lgs[:], in0=lgs[:], scalar1=lns[:, :1],
                                    scalar2=None,
                                    op0=mybir.AluOpType.subtract)
            nc.sync.dma_start(out=out[:, :], in_=lgs[:])
    nc.compile()
    _prog_cache[key] = nc
    return nc


# ----------------------------------------------------------------- execution
def _run(nc, in_maps):
    res = bass_utils.run_bass_kernel_spmd(nc, in_maps,
                                          core_ids=list(range(NCORES)),
                                          trace=TRACE)
    if TRACE:
        LAST_EXEC_NS.append(res.exec_time_ns)
    return res.results


def _tables_from_y(pl, ylocal):
    """ylocal: [NCORES, NLpad, F] -> 4 window tables [WROW+1, F] (shared
    logical row space; per-core identical)."""
    TLrows = NCORES * pl.NLpad
    flat = np.zeros((NW * (pl.WROW + 1), F), np.float32)
    full = ylocal.reshape(TLrows, F)
    for w in range(NW):
        lo = w * pl.WROW
        hi = min(lo + pl.WROW, TLrows)
        flat[w * (pl.WROW + 1):w * (pl.WROW + 1) + (hi - lo)] = full[lo:hi]
    return [flat[w * (pl.WROW + 1):(w + 1) * (pl.WROW + 1)] for w in range(NW)]


def _align_partials(pl, Sout):
    """Sout: [NCORES, STOT, F] window-ordered partial sums -> aligned
    [NW, NCORES, NLpad, F] (host permutation only)."""
    out = np.zeros((NW, NCORES, pl.NLpad, F), np.float32)
    # S rows -> (window, node) via call list
    for call in pl.calls:
        wi = call['w']
        rows = np.arange(call['sbase'], call['sbase'] + call['G'] * 128)
        for c in range(NCORES):
            nodes = pl.srow_node[c, rows]
            m = nodes >= 0
            out[wi, c, nodes[m]] = Sout[c, rows[m]]
    return out


def kernel(**inputs):
    x = np.asarray(inputs['x'], np.float32)
    edge_index = np.asarray(inputs['edge_index'])
    batch = np.asarray(inputs['batch'])
    N = x.shape[0]
    G = int(batch.max()) + 1
    pl = build_plan(edge_index, batch, N, G)

    # folded coefficient matrices (weights only)
    Wl = [np.asarray(inputs[f'Wl{i}'], np.float64) for i in range(3)]
    Wr = [np.asarray(inputs[f'Wr{i}'], np.float64) for i in range(3)]
    bl = [np.asarray(inputs[f'bl{i}'], np.float64) for i in range(3)]
    C0 = Wr[0] @ Wr[1] @ Wr[2]
    C1 = Wr[0] @ Wr[1] @ Wl[2] + Wr[0] @ Wl[1] @ Wr[2] + Wl[0] @ Wr[1] @ Wr[2]
    C2 = Wr[0] @ Wl[1] @ Wl[2] + Wl[0] @ Wr[1] @ Wl[2] + Wl[0] @ Wl[1] @ Wr[2]
    C3 = Wl[0] @ Wl[1] @ Wl[2]
    d0 = bl[0] @ Wr[1] @ Wr[2] + bl[1] @ Wr[2] + bl[2]
    d1 = bl[0] @ (Wr[1] @ Wl[2] + Wl[1] @ Wr[2]) + bl[1] @ Wl[2]
    d2 = bl[0] @ Wl[1] @ Wl[2]
    d3 = np.zeros(64)
    Cs = []
    for Cm, dv in [(C0, d0), (C1, d1), (C2, d2), (C3, d3)]:
        Cp = np.zeros((F, F), np.float32)
        Cp[:32] = Cm
        Cp[32] = dv
        Cs.append(Cp)
    Cstack = np.concatenate(Cs, axis=0)  # [4*64, 64]

    # y0 local: [NCORES, NLpad, 64], cols 0..31 = x, col 32 = 1
    y0 = np.zeros((NCORES, pl.NLpad, F), np.float32)
    for c in range(NCORES):
        nl = pl.NL[c]
        y0[c, :nl, :32] = x[pl.n0[c]:pl.n1[c]]
        y0[c, :nl, 32] = 1.0

    nc_agg = prog_agg(pl)
    nc_comb = prog_comb(pl)

    ys = [y0]
    ycur = y0
    for _ in range(3):
        tabs = _tables_from_y(pl, ycur)
        in_maps = []
        for c in range(NCORES):
            m = {f"tab{w}": tabs[w] for w in range(NW)}
            m["idx"] = pl.idxbuf[c]
            m["rec"] = pl.recbuf[c]
            in_maps.append(m)
        res = _run(nc_agg, in_maps)
        Sout = np.stack([res[c]["S"] for c in range(NCORES)])
        parts = _align_partials(pl, Sout)
        if len(ys) < 3:
            in_maps = [{f"S{w}": parts[w, c] for w in range(NW)}
                       for c in range(NCORES)]
            res = _run(nc_comb, in_maps)
            ycur = np.stack([res[c]["y"] for c in range(NCORES)])
            ys.append(ycur)
        else:
            last_parts = parts
            break

    # pooling plan: per core 8 graphs, padded node lists
    gpc = pl.gpc
    gsizes = np.bincount(np.asarray(batch), minlength=G)
    KG = int(_roundup(gsizes.max(), 16))  # slots per graph
    nslots = KG * 8
    pidx = np.full((NCORES, 128, nslots // 16), pl.NLpad, np.int16)
    batch_np = np.asarray(batch)
    for c in range(NCORES):
        Lp = np.full((nslots,), pl.NLpad, np.int16)
        for gi in range(gpc):
            gid = c * gpc + gi
            lo, hi = pl.n0[c], pl.n1[c]
            ln = np.nonzero(batch_np[lo:hi] == gid)[0]
            j = np.arange(len(ln))
            Lp[(j // 16) * 128 + gi * 16 + (j % 16)] = ln
        pidx[c] = np.tile(Lp.reshape(nslots // 16, 16).T, (8, 1))

    nc_tail = prog_tail(pl, KG)
    ninf = np.full((128, F), -1e30, np.float32)
    bo = np.asarray(inputs['b_out'], np.float32)[None, :].repeat(8, axis=0)
    Wo = np.zeros((F, 8), np.float32)
    Wo[:] = np.asarray(inputs['W_out'], np.float32)
    in_maps = []
    for c in range(NCORES):
        m = {f"S{w}": last_parts[w, c] for w in range(NW)}
        for k in range(3):
            m[f"y{k}"] = ys[k][c]
        m["C"] = Cstack
        m["Wo"] = Wo
        m["bo"] = bo
        m["ninf"] = ninf
        m["pidx"] = pidx[c]
        in_maps.append(m)
    res = _run(nc_tail, in_maps)

    out = np.zeros((G, 8), np.float32)
    for c in range(NCORES):
        out[c * gpc:(c + 1) * gpc] = res[c]["out"]
    return out



# revision 2
# speedup vs baseline: 1.3602x; 1.3602x over previous
"""Trainium2 Bass kernel for nn_ClusterNet (3-layer linear GraphSAGE + max-pool
+ log_softmax) — ap_gather edition.

The net is linear up to the final log_softmax:
    h3 = sum_{k=0..3} y_k @ C_k,  y_k = M^k x  (M = D^-1 A),
with the bias ride-along handled by structure-only host vectors u_k = M^k 1
(C_k folded on host from the small weight matrices, as in the prior kernel).

Device layout is feature-transposed: the global node table lives in SBUF as
[128 partitions, NE, 2] bf16 where partition 16*w + p holds feature pair
(2p, 2p+1) of src-window w (window w = NC w's nodes, in graph-slot order).
Each GPSIMD Q7 core k owns window k and expands that window's messages with
nc.gpsimd.ap_gather (SBUF->SBUF, no DMA descriptors — this removes the
SWDGE descriptor-generation bottleneck that dominated the dma_gather
kernel).  Per (NC, core) edge streams share one rank-based slot template so
every DVE tree-add instruction is SPMD-uniform; a second small ap_gather
reorders each core's bucket-ordered partial back to slot order, a PE matmul
folds the 8 windows across partitions, and a DVE multiply applies 1/deg.
Nodes use a graph-slot layout (graph g padded to GP slots) so the final
max-pool is 8 fixed-range reductions.  3 launches: y1, y2, y3+tail; the
host only permutes bytes between launches (table assembly).
"""
import os
import sys

sys.path.insert(0, '/opt/trn_rl_repo')

import numpy as np
import ml_dtypes

import concourse.bass as bass
import concourse.bacc as bacc
import concourse.tile as tile
import concourse.mybir as mybir
from concourse import bass_utils

NCORES = 8
BUCKETS = [1, 2, 3, 4, 5, 6, 7, 8, 9, 10, 12, 14, 16, 20, 24, 28, 32, 40,
           48, 56, 64]
CH_G = 2048     # gather slots per ap_gather call
CH_R = 2048     # reorder idxs per ap_gather call
FOLD = 256      # nodes per fold matmul (512 f32 PSUM)
TRACE = bool(os.environ.get("KERNEL_TRACE"))
SIM = bool(os.environ.get("KERNEL_SIM"))

LAST_EXEC_NS = []

if TRACE and not SIM:
    import types

    if "antenv.axon_hooks" not in sys.modules:
        _m = types.ModuleType("antenv.axon_hooks")
        _m._hook = None
        _m.set_axon_ntff_profile_hook = lambda h: setattr(_m, "_hook", h)
        _m.get_axon_ntff_profile_hook = lambda: _m._hook
        sys.modules["antenv.axon_hooks"] = _m
        try:
            from trn_agent_boot.trn_boot import _ntff_profile_via_ctypes
            _m._hook = _ntff_profile_via_ctypes("/opt/axon/libaxon_pjrt.so")
        except Exception:
            _m._hook = None
    bass_utils.upload_artifacts = lambda tmpdir: f"local:{tmpdir}"

_prog_cache = {}
BF16 = ml_dtypes.bfloat16


def _roundup(a, b):
    return (a + b - 1) // b * b


class Plan:
    pass


def build_plan(edge_index, batch, N, G):
    pl = Plan()
    src = np.asarray(edge_index[0], np.int64)
    dst = np.asarray(edge_index[1], np.int64)
    batch = np.asarray(batch, np.int64)
    gpc = G // NCORES
    pl.gpc = gpc

    gstart = np.searchsorted(batch, np.arange(G))
    gsz = np.bincount(batch, minlength=G)
    GP = _roundup(int(gsz.max()), 32)
    NLR = gpc * GP
    NE = NLR + 1
    assert NE * 4 <= 131072 and NE - 1 <= 32767
    assert NLR % 16 == 0
    pl.GP, pl.NLR, pl.NE = GP, NLR, NE

    node_nc = batch // gpc                                   # NC of node
    slotpos = (batch % gpc) * GP + (np.arange(N) - gstart[batch])  # [N]
    pl.slotpos = slotpos
    pl.node_nc = node_nc
    # slot -> node map per NC (-1 = padding)
    nodeat = np.full((NCORES, NLR), -1, np.int64)
    nodeat[node_nc, slotpos] = np.arange(N)
    pl.nodeat = nodeat

    deg = np.bincount(dst, minlength=N)
    recip = np.where(deg > 0, 1.0 / np.maximum(deg, 1), 0.0)
    pl.deg, pl.recip = deg, recip

    enc = node_nc[dst]
    ew = node_nc[src]

    # ---- per-stream (nc, w) rank lists --------------------------------
    # stream key sorts edges by (nc, w, dst)
    order = np.lexsort((dst, ew, enc))
    s_nc, s_w, s_dst, s_src = enc[order], ew[order], dst[order], src[order]
    # group by (nc, w, dst)
    key = (s_nc * NCORES + s_w) * N + s_dst
    uniq_key, grp_start, grp_cnt = np.unique(key, return_index=True,
                                             return_counts=True)
    g_nc = uniq_key // (NCORES * N)
    g_w = (uniq_key // N) % NCORES
    g_dst = uniq_key % N

    # per stream: ranks sorted by count desc (stable)
    streams = {}
    L_T = 0
    rank_counts = []
    for c in range(NCORES):
        for w in range(NCORES):
            m = (g_nc == c) & (g_w == w)
            cnt = grp_cnt[m]
            so = np.argsort(-cnt, kind='stable')
            streams[(c, w)] = (g_dst[m][so], cnt[so], grp_start[m][so])
            L_T = max(L_T, cnt.size)
            rank_counts.append(cnt[so])
    rank_max = np.zeros(L_T, np.int64)
    for rc in rank_counts:
        rank_max[:rc.size] = np.maximum(rank_max[:rc.size], rc)
    bidx = np.searchsorted(BUCKETS, rank_max)
    assert bidx.max() < len(BUCKETS)
    T = np.asarray(BUCKETS, np.int64)[bidx]          # slots per rank
    pl.L_T = L_T
    assert (L_T + 1) <= 16384 and (L_T + 1) * 4 <= 131072

    # segments: runs of equal T
    segs = []
    j = 0
    while j < L_T:
        k = int(T[j])
        j2 = j
        while j2 < L_T and T[j2] == k:
            j2 += 1
        segs.append((k, j, j2 - j))
        j = j2

    # chunks of CH_G slots; pieces = (K, rank0, nranks, slot_off)
    chunks = []
    cur, slot = [], 0
    for (k, r0, n) in segs:
        left = n
        rr = r0
        while left > 0:
            fit = min(left, (CH_G - slot) // k)
            if fit == 0:
                chunks.append(cur)
                cur, slot = [], 0
                continue
            cur.append((k, rr, fit, slot))
            slot += fit * k
            rr += fit
            left -= fit
    if cur:
        chunks.append(cur)
    pl.chunks = chunks
    pl.SG = len(chunks) * CH_G
    pl.chunk_fill = [
        _roundup(max(soff + n * k for (k, r0, n, soff) in ch), 16)
        for ch in chunks]

    # rank -> (chunk, slot base, K)
    rank_chunk = np.zeros(L_T, np.int64)
    rank_base = np.zeros(L_T, np.int64)
    rank_K = np.zeros(L_T, np.int64)
    for ci, ch in enumerate(chunks):
        for (k, r0, n, soff) in ch:
            jj = np.arange(n)
            rank_chunk[r0:r0 + n] = ci
            rank_base[r0:r0 + n] = soff + jj * k
            rank_K[r0:r0 + n] = k

    # ---- per-NC device input buffers ----------------------------------
    zero_idx = NE - 1
    ZC = L_T                     # zero column in compact array
    idxg = np.full((NCORES, NCORES, pl.SG), zero_idx, np.int16)  # [nc][core]
    idxr = np.full((NCORES, NCORES, NLR), ZC, np.int16)
    src_slot = slotpos[s_src]    # slot-space src of each sorted edge
    for c in range(NCORES):
        for w in range(NCORES):
            nodes_r, cnt_r, gs_r = streams[(c, w)]
            base = rank_chunk[:cnt_r.size] * CH_G + rank_base[:cnt_r.size]
            tot = int(cnt_r.sum())
            jj = np.arange(tot)
            rep = np.repeat(np.arange(cnt_r.size), cnt_r)
            off0 = np.concatenate([[0], np.cumsum(cnt_r)[:-1]])
            within = jj - off0[rep]
            pos = base[rep] + within
            vals = src_slot[np.repeat(gs_r, cnt_r) + within]
            idxg[c, w, pos] = vals
            idxr[c, w, slotpos[nodes_r]] = np.arange(cnt_r.size, dtype=np.int16)
    pl.idxg_w = np.zeros((NCORES, 128, pl.SG // 16), np.int16)
    pl.idxr_w = np.zeros((NCORES, 128, NLR // 16), np.int16)
    for c in range(NCORES):
        for w in range(NCORES):
            pl.idxg_w[c, 16 * w:16 * w + 16] = \
                idxg[c, w].reshape(pl.SG // 16, 16).T
            pl.idxr_w[c, 16 * w:16 * w + 16] = \
                idxr[c, w].reshape(NLR // 16, 16).T

    # recip / mask / u in slot order
    pl.recip_sl = np.zeros((NCORES, NLR), np.float32)
    pl.mask_sl = np.full((NCORES, NLR), -1e30, np.float32)
    for c in range(NCORES):
        m = nodeat[c] >= 0
        pl.recip_sl[c, m] = recip[nodeat[c, m]]
        pl.mask_sl[c, m] = 0.0

    # u_k = M^k 1 (structure only)
    u = np.zeros((4, N))
    u[0] = 1.0
    for k in range(3):
        s = np.bincount(dst, weights=u[k][src], minlength=N)
        u[k + 1] = recip * s
    pl.u_sl = np.zeros((NCORES, 4, NLR), np.float32)
    for c in range(NCORES):
        m = nodeat[c] >= 0
        pl.u_sl[c][:, m] = u[:, nodeat[c, m]]

    # fold matrix
    foldF = np.zeros((128, 16), np.float32)
    for c in range(NCORES):
        foldF[16 * c + np.arange(16), np.arange(16)] = 1.0
    pl.foldF = foldF.astype(BF16)
    return pl


def make_table(pl, yplanes):
    """yplanes: [NCORES, 2, 16, NLR] bf16 (uint16 view ok) -> tab
    [128, NE*2] bf16 per-NC-identical global table (byte permutation only)."""
    tab = np.zeros((128, pl.NE, 2), np.uint16)
    yv = yplanes.view(np.uint16)
    for w in range(NCORES):
        tab[16 * w:16 * w + 16, :pl.NLR, 0] = yv[w, 0]
        tab[16 * w:16 * w + 16, :pl.NLR, 1] = yv[w, 1]
    return tab.reshape(128, pl.NE * 2).view(BF16)


# ----------------------------------------------------------- device program
def _emit_agg(nc, tc, pl, pools, tab_t, idxg_t, idxr_t, recip_t, foldF_t,
              y_dram0, y_dram1):
    gp, rp, yp, psp, ap_ = pools
    f32 = mybir.dt.float32
    bf = mybir.dt.bfloat16

    ct = ap_.tile([128, (pl.L_T + 1) * 2], bf)
    ctv = ct[:].rearrange("p (e d) -> p e d", d=2)
    nc.vector.memset(ctv[:, pl.L_T, :], 0.0)
    tabv = tab_t[:].rearrange("p (e d) -> p e d", d=2)
    for ci, ch in enumerate(pl.chunks):
        fill = pl.chunk_fill[ci]
        got = gp.tile([128, CH_G * 2], bf, tag="got")
        gv = got[:].rearrange("p (i d) -> p i d", d=2)
        nc.gpsimd.ap_gather(
            out_ap=gv[:, :fill, :], in_ap=tabv,
            idxs_ap=idxg_t[:, ci * CH_G // 16:ci * CH_G // 16 + fill // 16],
            channels=128, num_elems=pl.NE, d=2, num_idxs=fill)
        for (K, r0, nr, soff) in ch:
            pv = gv[:, soff:soff + nr * K, :].rearrange(
                "p (n k) d -> p n k d", k=K)
            kk = K
            while kk > 1:
                h = kk // 2
                nc.vector.tensor_add(out=pv[:, :, :h, :], in0=pv[:, :, :h, :],
                                     in1=pv[:, :, h:2 * h, :])
                if kk % 2 == 1:
                    nc.vector.tensor_add(out=pv[:, :, 0, :],
                                         in0=pv[:, :, 0, :],
                                         in1=pv[:, :, kk - 1, :])
                kk = h
            nc.vector.tensor_copy(out=ctv[:, r0:r0 + nr, :],
                                  in_=pv[:, :, 0, :])
    # reorder to slot order + fold across windows + scale by recip
    for rc in range((pl.NLR + CH_R - 1) // CH_R):
        ncols = min(CH_R, pl.NLR - rc * CH_R)
        rot = rp.tile([128, CH_R * 2], bf, tag="rot")
        rv = rot[:].rearrange("p (i d) -> p i d", d=2)
        nc.gpsimd.ap_gather(
            out_ap=rv[:, :ncols, :], in_ap=ctv,
            idxs_ap=idxr_t[:, rc * CH_R // 16:rc * CH_R // 16 + ncols // 16],
            channels=128, num_elems=pl.L_T + 1, d=2, num_idxs=ncols)
        yst0 = yp.tile([16, CH_R], bf, tag="y0")
        yst1 = yp.tile([16, CH_R], bf, tag="y1")
        for f0 in range(0, ncols, FOLD):
            fw = min(FOLD, ncols - f0)
            ps = psp.tile([16, FOLD * 2], f32, space="PSUM", tag="ps")
            nc.tensor.matmul(out=ps[:, :fw * 2], lhsT=foldF_t[:],
                             rhs=rot[:, f0 * 2:(f0 + fw) * 2],
                             start=True, stop=True)
            psv = ps[:].rearrange("p (n d) -> p n d", d=2)
            cols = slice(rc * CH_R + f0, rc * CH_R + f0 + fw)
            nc.vector.tensor_mul(out=yst0[:, f0:f0 + fw], in0=psv[:, :fw, 0],
                                 in1=recip_t[:, cols])
            nc.vector.tensor_mul(out=yst1[:, f0:f0 + fw], in0=psv[:, :fw, 1],
                                 in1=recip_t[:, cols])
        nc.sync.dma_start(out=y_dram0[:, rc * CH_R:rc * CH_R + ncols],
                          in_=yst0[:, :ncols])
        nc.sync.dma_start(out=y_dram1[:, rc * CH_R:rc * CH_R + ncols],
                          in_=yst1[:, :ncols])


def prog(pl, tail):
    key = ('prog', tail, pl.NE, pl.L_T, pl.SG, pl.NLR, tuple(pl.chunk_fill))
    if key in _prog_cache:
        return _prog_cache[key]
    nc = bacc.Bacc("TRN2", target_bir_lowering=False, debug=False,
                   num_devices=NCORES)
    f32 = mybir.dt.float32
    f16 = mybir.dt.float16
    bf = mybir.dt.bfloat16
    i16 = mybir.dt.int16

    tab_d = nc.dram_tensor("tab", (128, pl.NE * 2), bf,
                           kind="ExternalInput").ap()
    idxg_d = nc.dram_tensor("idxg", (128, pl.SG // 16), i16,
                            kind="ExternalInput").ap()
    idxr_d = nc.dram_tensor("idxr", (128, pl.NLR // 16), i16,
                            kind="ExternalInput").ap()
    recip_d = nc.dram_tensor("recip", (16, pl.NLR), f16,
                             kind="ExternalInput").ap()
    foldF_d = nc.dram_tensor("foldF", (128, 16), bf,
                             kind="ExternalInput").ap()
    if not tail:
        y0_d = nc.dram_tensor("yj0", (16, pl.NLR), bf,
                              kind="ExternalOutput").ap()
        y1_d = nc.dram_tensor("yj1", (16, pl.NLR), bf,
                              kind="ExternalOutput").ap()
    else:
        y0_d = nc.dram_tensor("y3t0", (16, pl.NLR), bf, kind="Internal").ap()
        y1_d = nc.dram_tensor("y3t1", (16, pl.NLR), bf, kind="Internal").ap()
        yin0_d = nc.dram_tensor("yin0", (48, pl.NLR), bf,
                                kind="ExternalInput").ap()
        yin1_d = nc.dram_tensor("yin1", (48, pl.NLR), bf,
                                kind="ExternalInput").ap()
        u_d = nc.dram_tensor("u", (4, pl.NLR), bf, kind="ExternalInput").ap()
        mask_d = nc.dram_tensor("maskv", (1, pl.NLR), bf,
                                kind="ExternalInput").ap()
        ct_d = nc.dram_tensor("CT", (128, 128), bf, kind="ExternalInput").ap()
        wo_d = nc.dram_tensor("Wo", (64, 8), f32, kind="ExternalInput").ap()
        bo_d = nc.dram_tensor("bo", (8, 8), f32, kind="ExternalInput").ap()
        out_d = nc.dram_tensor("out", (8, 8), f32, kind="ExternalOutput").ap()

    with tile.TileContext(nc) as tc:
        with tc.tile_pool(name="persist", bufs=1) as pp:
            idxg_t = pp.tile([128, pl.SG // 16], i16)
            nc.sync.dma_start(out=idxg_t[:], in_=idxg_d[:, :])
            idxr_t = pp.tile([128, pl.NLR // 16], i16)
            nc.sync.dma_start(out=idxr_t[:], in_=idxr_d[:, :])
            recip_t = pp.tile([16, pl.NLR], f16)
            nc.sync.dma_start(out=recip_t[:], in_=recip_d[:, :])
            foldF_t = pp.tile([128, 16], bf)
            nc.sync.dma_start(out=foldF_t[:], in_=foldF_d[:, :])

            with tc.tile_pool(name="agg", bufs=1) as ap_, \
                 tc.tile_pool(name="g", bufs=2) as gp, \
                 tc.tile_pool(name="r", bufs=2) as rp, \
                 tc.tile_pool(name="ys", bufs=2) as yp, \
                 tc.tile_pool(name="ps", bufs=4, space="PSUM") as psp:
                tab_t = ap_.tile([128, pl.NE * 2], bf)
                nc.sync.dma_start(out=tab_t[:], in_=tab_d[:, :])
                _emit_agg(nc, tc, pl, (gp, rp, yp, psp, ap_), tab_t, idxg_t,
                          idxr_t, recip_t, foldF_t, y0_d, y1_d)

            if tail:
                with tc.tile_pool(name="tail", bufs=1) as tp, \
                     tc.tile_pool(name="ps2", bufs=4, space="PSUM") as psp2:
                    Y0 = tp.tile([128, pl.NLR], bf)
                    Y1 = tp.tile([128, pl.NLR], bf)
                    nc.vector.memset(Y0[:], 0.0)
                    nc.vector.memset(Y1[:], 0.0)
                    # y3 at rows 0-15 (round trip via internal dram)
                    nc.sync.dma_start(out=Y0[0:16, :], in_=y0_d[:, :])
                    nc.sync.dma_start(out=Y1[0:16, :], in_=y1_d[:, :])
                    # y0,y1,y2 at rows 32/64/96
                    for i in range(3):
                        nc.sync.dma_start(out=Y0[32 * (i + 1):32 * (i + 1) + 16, :],
                                          in_=yin0_d[16 * i:16 * i + 16, :])
                        nc.sync.dma_start(out=Y1[32 * (i + 1):32 * (i + 1) + 16, :],
                                          in_=yin1_d[16 * i:16 * i + 16, :])
                    # u rows: layer k block row +16 (block order y3,y0,y1,y2)
                    for blk, k in ((0, 3), (1, 0), (2, 1), (3, 2)):
                        nc.sync.dma_start(out=Y0[32 * blk + 16:32 * blk + 17, :],
                                          in_=u_d[k:k + 1, :])
                        nc.sync.dma_start(out=Y1[32 * blk + 16:32 * blk + 17, :],
                                          in_=u_d[k:k + 1, :])
                    # mask row 17 (plane 0 only)
                    nc.sync.dma_start(out=Y0[17:18, :], in_=mask_d[:, :])
                    CT_t = tp.tile([128, 128], bf)
                    nc.sync.dma_start(out=CT_t[:], in_=ct_d[:, :])
                    h3sb = tp.tile([64, pl.NLR], bf)
                    for f0 in range(0, pl.NLR, FOLD):
                        fw = min(FOLD, pl.NLR - f0)
                        hp = psp2.tile([64, FOLD], f32, space="PSUM", tag="hp")
                        nc.tensor.matmul(out=hp[:, :fw], lhsT=CT_t[:, 0:64],
                                         rhs=Y0[:, f0:f0 + fw],
                                         start=True, stop=False)
                        nc.tensor.matmul(out=hp[:, :fw], lhsT=CT_t[:, 64:128],
                                         rhs=Y1[:, f0:f0 + fw],
                                         start=False, stop=True)
                        nc.scalar.copy(out=h3sb[:, f0:f0 + fw], in_=hp[:, :fw])
                    pooled = tp.tile([64, 8], f32)
                    for g in range(8):
                        nc.vector.tensor_reduce(
                            out=pooled[:, g:g + 1],
                            in_=h3sb[:, g * pl.GP:(g + 1) * pl.GP],
                            axis=mybir.AxisListType.X, op=mybir.AluOpType.max)
                    wo_t = tp.tile([64, 8], f32)
                    nc.sync.dma_start(out=wo_t[:], in_=wo_d[:, :])
                    bo_t = tp.tile([8, 8], f32)
                    nc.sync.dma_start(out=bo_t[:], in_=bo_d[:, :])
                    lg = psp2.tile([8, 8], f32, space="PSUM", tag="lg")
                    nc.tensor.matmul(out=lg[:], lhsT=pooled[:], rhs=wo_t[:],
                                     start=True, stop=True)
                    lgs = tp.tile([8, 8], f32)
                    nc.vector.tensor_add(out=lgs[:], in0=lg[:], in1=bo_t[:])
                    mx = tp.tile([8, 1], f32)
                    nc.vector.tensor_reduce(out=mx[:], in_=lgs[:],
                                            axis=mybir.AxisListType.X,
                                            op=mybir.AluOpType.max)
                    nc.vector.tensor_scalar(out=lgs[:], in0=lgs[:],
                                            scalar1=mx[:, :1], scalar2=None,
                                            op0=mybir.AluOpType.subtract)
                    ex = tp.tile([8, 8], f32)
                    nc.scalar.activation(out=ex[:], in_=lgs[:],
                                         func=mybir.ActivationFunctionType.Exp)
                    sm = tp.tile([8, 1], f32)
                    nc.vector.tensor_reduce(out=sm[:], in_=ex[:],
                                            axis=mybir.AxisListType.X,
                                            op=mybir.AluOpType.add)
                    lns = tp.tile([8, 1], f32)
                    nc.scalar.activation(out=lns[:], in_=sm[:],
                                         func=mybir.ActivationFunctionType.Ln)
                    nc.vector.tensor_scalar(out=lgs[:], in0=lgs[:],
                                            scalar1=lns[:, :1], scalar2=None,
                                            op0=mybir.AluOpType.subtract)
                    nc.sync.dma_start(out=out_d[:, :], in_=lgs[:])
    nc.compile()
    _prog_cache[key] = nc
    return nc


# ----------------------------------------------------------------- execution
def _run(nc, in_maps, outputs):
    if SIM:
        import concourse.bass_interp as bi
        sim = bi.MultiCoreSim(nc, num_cores=NCORES)
        for cid, cs in sim.cores.items():
            for k, v in in_maps[cid].items():
                cs.tensor(k)[:] = v
        sim.simulate()
        return [{o: np.array(sim.cores[c].tensor(o)) for o in outputs}
                for c in range(NCORES)]
    res = bass_utils.run_bass_kernel_spmd(nc, in_maps,
                                          core_ids=list(range(NCORES)),
                                          trace=TRACE)
    if TRACE:
        LAST_EXEC_NS.append(res.exec_time_ns)
    return res.results


def kernel(**inputs):
    x = np.asarray(inputs['x'], np.float32)
    edge_index = np.asarray(inputs['edge_index'])
    batch = np.asarray(inputs['batch'])
    N = x.shape[0]
    G = int(batch.max()) + 1
    F_IN = x.shape[1]
    pl = build_plan(edge_index, batch, N, G)

    # folded coefficients (host, weights only — same as prior kernel)
    Wl = [np.asarray(inputs[f'Wl{i}'], np.float64) for i in range(3)]
    Wr = [np.asarray(inputs[f'Wr{i}'], np.float64) for i in range(3)]
    bl = [np.asarray(inputs[f'bl{i}'], np.float64) for i in range(3)]
    C0 = Wr[0] @ Wr[1] @ Wr[2]
    C1 = Wr[0] @ Wr[1] @ Wl[2] + Wr[0] @ Wl[1] @ Wr[2] + Wl[0] @ Wr[1] @ Wr[2]
    C2 = Wr[0] @ Wl[1] @ Wl[2] + Wl[0] @ Wr[1] @ Wl[2] + Wl[0] @ Wl[1] @ Wr[2]
    C3 = Wl[0] @ Wl[1] @ Wl[2]
    d0 = bl[0] @ Wr[1] @ Wr[2] + bl[1] @ Wr[2] + bl[2]
    d1 = bl[0] @ (Wr[1] @ Wl[2] + Wl[1] @ Wr[2]) + bl[1] @ Wl[2]
    d2 = bl[0] @ Wl[1] @ Wl[2]
    d3 = np.zeros(64)
    Cs = {0: (C0, d0), 1: (C1, d1), 2: (C2, d2), 3: (C3, d3)}
    # CT[j][128, 64]: partition block 32*blk (blk order: y3,y0,y1,y2)
    CT = np.zeros((2, 128, 64), np.float32)
    for blk, k in ((0, 3), (1, 0), (2, 1), (3, 2)):
        Cm, dv = Cs[k]
        for j in range(2):
            rows = 2 * np.arange(16) + j
            CT[j, 32 * blk + np.arange(16)] = Cm[rows]
            CT[j, 32 * blk + 16] = dv / 2.0
    CT[0, 17] = 1.0  # mask row (plane 0)
    CT_in = np.concatenate([CT[0], CT[1]], axis=1).astype(BF16)  # [128,128]

    # initial table from x (bf16, slot order, feature-pair layout)
    xb = x.astype(BF16).view(np.uint16)
    y0planes = np.zeros((NCORES, 2, 16, pl.NLR), np.uint16)
    for c in range(NCORES):
        m = pl.nodeat[c] >= 0
        nodes = pl.nodeat[c, m]
        for j in range(2):
            # plane j, partition p = feature 2p+j
            y0planes[c, j][:, m] = xb[nodes][:, 2 * np.arange(16) + j].T
    tab0 = make_table(pl, y0planes.view(BF16))

    recip16 = pl.recip_sl.astype(np.float16)
    statics = []
    for c in range(NCORES):
        statics.append({
            "idxg": pl.idxg_w[c],
            "idxr": pl.idxr_w[c],
            "recip": np.broadcast_to(recip16[c][None, :], (16, pl.NLR)).copy(),
            "foldF": pl.foldF,
        })

    nc_agg = prog(pl, tail=False)
    yplanes = [y0planes]
    tab = tab0
    for layer in range(2):
        in_maps = [dict(statics[c], tab=tab) for c in range(NCORES)]
        res = _run(nc_agg, in_maps, ["yj0", "yj1"])
        yp = np.zeros((NCORES, 2, 16, pl.NLR), np.uint16)
        for c in range(NCORES):
            yp[c, 0] = np.asarray(res[c]["yj0"]).view(np.uint16)
            yp[c, 1] = np.asarray(res[c]["yj1"]).view(np.uint16)
        yplanes.append(yp)
        tab = make_table(pl, yp.view(BF16))

    nc_tail = prog(pl, tail=True)
    in_maps = []
    for c in range(NCORES):
        m = dict(statics[c], tab=tab)
        yin0 = np.concatenate([yplanes[k][c, 0] for k in range(3)], axis=0)
        yin1 = np.concatenate([yplanes[k][c, 1] for k in range(3)], axis=0)
        m["yin0"] = yin0.view(BF16)
        m["yin1"] = yin1.view(BF16)
        m["u"] = pl.u_sl[c].astype(BF16)
        m["maskv"] = pl.mask_sl[c][None, :].astype(BF16)
        m["CT"] = CT_in
        m["Wo"] = np.asarray(inputs['W_out'], np.float32)
        m["bo"] = np.asarray(inputs['b_out'], np.float32)[None, :].repeat(8, 0)
        in_maps.append(m)
    res = _run(nc_tail, in_maps, ["out"])

    out = np.zeros((G, 8), np.float32)
    for c in range(NCORES):
        out[c * pl.gpc:(c + 1) * pl.gpc] = res[c]["out"]
    return out


# revision 3
# speedup vs baseline: 1.3719x; 1.0086x over previous
"""Trainium2 Bass kernel for nn_ClusterNet (3-layer linear GraphSAGE + max-pool
+ log_softmax) — ap_gather edition.

The net is linear up to the final log_softmax:
    h3 = sum_{k=0..3} y_k @ C_k,  y_k = M^k x  (M = D^-1 A),
with the bias ride-along handled by structure-only host vectors u_k = M^k 1
(C_k folded on host from the small weight matrices, as in the prior kernel).

Device layout is feature-transposed: the global node table lives in SBUF as
[128 partitions, NE, 2] bf16 where partition 16*w + p holds feature pair
(2p, 2p+1) of src-window w (window w = NC w's nodes, in graph-slot order).
Each GPSIMD Q7 core k owns window k and expands that window's messages with
nc.gpsimd.ap_gather (SBUF->SBUF, no DMA descriptors — this removes the
SWDGE descriptor-generation bottleneck that dominated the dma_gather
kernel).  Per (NC, core) edge streams share one rank-based slot template so
every DVE tree-add instruction is SPMD-uniform; a second small ap_gather
reorders each core's bucket-ordered partial back to slot order, a PE matmul
folds the 8 windows across partitions, and a DVE multiply applies 1/deg.
Nodes use a graph-slot layout (graph g padded to GP slots) so the final
max-pool is 8 fixed-range reductions.  3 launches: y1, y2, y3+tail; the
host only permutes bytes between launches (table assembly).
"""
import os
import sys

sys.path.insert(0, '/opt/trn_rl_repo')

import numpy as np
import ml_dtypes

import concourse.bass as bass
import concourse.bacc as bacc
import concourse.tile as tile
import concourse.mybir as mybir
from concourse import bass_utils

NCORES = 8
NW = 4          # src windows (2 NCs each); cores pair as (window, dst-half)
BUCKETS = [1, 2, 3, 4, 5, 6, 7, 8, 9, 10, 12, 14, 16, 20, 24, 28, 32, 40,
           48, 56, 64, 80, 96, 128]
CH_G = 1024     # gather slots per ap_gather call
CH_R = 2048     # reorder idxs per ap_gather call
FOLD = 256      # nodes per fold matmul (512 f32 PSUM)
TRACE = bool(os.environ.get("KERNEL_TRACE"))
SIM = bool(os.environ.get("KERNEL_SIM"))

LAST_EXEC_NS = []

if TRACE and not SIM:
    import types

    if "antenv.axon_hooks" not in sys.modules:
        _m = types.ModuleType("antenv.axon_hooks")
        _m._hook = None
        _m.set_axon_ntff_profile_hook = lambda h: setattr(_m, "_hook", h)
        _m.get_axon_ntff_profile_hook = lambda: _m._hook
        sys.modules["antenv.axon_hooks"] = _m
        try:
            from trn_agent_boot.trn_boot import _ntff_profile_via_ctypes
            _m._hook = _ntff_profile_via_ctypes("/opt/axon/libaxon_pjrt.so")
        except Exception:
            _m._hook = None
    bass_utils.upload_artifacts = lambda tmpdir: f"local:{tmpdir}"

_prog_cache = {}
BF16 = ml_dtypes.bfloat16


def _roundup(a, b):
    return (a + b - 1) // b * b


class Plan:
    pass


def build_plan(edge_index, batch, N, G):
    pl = Plan()
    src = np.asarray(edge_index[0], np.int64)
    dst = np.asarray(edge_index[1], np.int64)
    batch = np.asarray(batch, np.int64)
    gpc = G // NCORES
    pl.gpc = gpc

    gstart = np.searchsorted(batch, np.arange(G))
    gsz = np.bincount(batch, minlength=G)
    GP = _roundup(int(gsz.max()), 32)
    NLR = gpc * GP
    NE = 2 * NLR + 1            # window = 2 NCs' nodes + zero col
    HGP = NLR // 2              # dst-half size (4 graphs)
    assert NE * 4 <= 131072 and NE - 1 <= 32767
    assert NLR % 16 == 0 and HGP % 16 == 0
    pl.GP, pl.NLR, pl.NE, pl.HGP = GP, NLR, NE, HGP

    node_nc = batch // gpc                                   # NC of node
    slotpos = (batch % gpc) * GP + (np.arange(N) - gstart[batch])  # [N]
    pl.slotpos = slotpos
    pl.node_nc = node_nc
    # slot -> node map per NC (-1 = padding)
    nodeat = np.full((NCORES, NLR), -1, np.int64)
    nodeat[node_nc, slotpos] = np.arange(N)
    pl.nodeat = nodeat

    deg = np.bincount(dst, minlength=N)
    recip = np.where(deg > 0, 1.0 / np.maximum(deg, 1), 0.0)
    pl.deg, pl.recip = deg, recip

    enc = node_nc[dst]
    # core of an edge: 2*src_window + dst_half
    ek = 2 * (node_nc[src] // 2) + (slotpos[dst] >= HGP).astype(np.int64)

    # ---- per-stream (nc, core) rank lists -----------------------------
    order = np.lexsort((dst, ek, enc))
    s_nc, s_w, s_dst, s_src = enc[order], ek[order], dst[order], src[order]
    # group by (nc, core, dst)
    key = (s_nc * NCORES + s_w) * N + s_dst
    uniq_key, grp_start, grp_cnt = np.unique(key, return_index=True,
                                             return_counts=True)
    g_nc = uniq_key // (NCORES * N)
    g_w = (uniq_key // N) % NCORES
    g_dst = uniq_key % N

    # per stream: ranks sorted by count desc (stable)
    streams = {}
    L_T = 0
    rank_counts = []
    for c in range(NCORES):
        for w in range(NCORES):
            m = (g_nc == c) & (g_w == w)
            cnt = grp_cnt[m]
            so = np.argsort(-cnt, kind='stable')
            streams[(c, w)] = (g_dst[m][so], cnt[so], grp_start[m][so])
            L_T = max(L_T, cnt.size)
            rank_counts.append(cnt[so])
    rank_max = np.zeros(L_T, np.int64)
    for rc in rank_counts:
        rank_max[:rc.size] = np.maximum(rank_max[:rc.size], rc)
    bidx = np.searchsorted(BUCKETS, rank_max)
    assert bidx.max() < len(BUCKETS)
    T = np.asarray(BUCKETS, np.int64)[bidx]          # slots per rank
    pl.L_T = L_T
    assert (L_T + 1) <= 16384 and (L_T + 1) * 4 <= 131072

    # segments: runs of equal T
    segs = []
    j = 0
    while j < L_T:
        k = int(T[j])
        j2 = j
        while j2 < L_T and T[j2] == k:
            j2 += 1
        segs.append((k, j, j2 - j))
        j = j2

    # chunks of CH_G slots; pieces = (K, rank0, nranks, slot_off)
    chunks = []
    cur, slot = [], 0
    for (k, r0, n) in segs:
        left = n
        rr = r0
        while left > 0:
            fit = min(left, (CH_G - slot) // k)
            if fit == 0:
                chunks.append(cur)
                cur, slot = [], 0
                continue
            cur.append((k, rr, fit, slot))
            slot += fit * k
            rr += fit
            left -= fit
    if cur:
        chunks.append(cur)
    pl.chunks = chunks
    pl.SG = len(chunks) * CH_G
    pl.chunk_fill = [
        _roundup(max(soff + n * k for (k, r0, n, soff) in ch), 16)
        for ch in chunks]

    # rank -> (chunk, slot base, K)
    rank_chunk = np.zeros(L_T, np.int64)
    rank_base = np.zeros(L_T, np.int64)
    rank_K = np.zeros(L_T, np.int64)
    for ci, ch in enumerate(chunks):
        for (k, r0, n, soff) in ch:
            jj = np.arange(n)
            rank_chunk[r0:r0 + n] = ci
            rank_base[r0:r0 + n] = soff + jj * k
            rank_K[r0:r0 + n] = k

    # ---- per-NC device input buffers ----------------------------------
    zero_idx = NE - 1
    ZC = L_T                     # zero column in compact array
    idxg = np.full((NCORES, NCORES, pl.SG), zero_idx, np.int16)  # [nc][core]
    idxr = np.full((NCORES, NCORES, HGP), ZC, np.int16)
    # slot-space src within its 2-NC window
    src_slot = (node_nc[s_src] % 2) * NLR + slotpos[s_src]
    for c in range(NCORES):
        for w in range(NCORES):
            nodes_r, cnt_r, gs_r = streams[(c, w)]
            base = rank_chunk[:cnt_r.size] * CH_G + rank_base[:cnt_r.size]
            tot = int(cnt_r.sum())
            jj = np.arange(tot)
            rep = np.repeat(np.arange(cnt_r.size), cnt_r)
            off0 = np.concatenate([[0], np.cumsum(cnt_r)[:-1]])
            within = jj - off0[rep]
            pos = base[rep] + within
            vals = src_slot[np.repeat(gs_r, cnt_r) + within]
            idxg[c, w, pos] = vals
            idxr[c, w, slotpos[nodes_r] - (w % 2) * HGP] = \
                np.arange(cnt_r.size, dtype=np.int16)
    pl.idxg_w = np.zeros((NCORES, 128, pl.SG // 16), np.int16)
    pl.idxr_w = np.zeros((NCORES, 128, HGP // 16), np.int16)
    for c in range(NCORES):
        for w in range(NCORES):
            pl.idxg_w[c, 16 * w:16 * w + 16] = \
                idxg[c, w].reshape(pl.SG // 16, 16).T
            pl.idxr_w[c, 16 * w:16 * w + 16] = \
                idxr[c, w].reshape(HGP // 16, 16).T

    # recip / mask / u in slot order
    pl.recip_sl = np.zeros((NCORES, NLR), np.float32)
    pl.mask_sl = np.full((NCORES, NLR), -1e30, np.float32)
    for c in range(NCORES):
        m = nodeat[c] >= 0
        pl.recip_sl[c, m] = recip[nodeat[c, m]]
        pl.mask_sl[c, m] = 0.0

    # u_k = M^k 1 (structure only)
    u = np.zeros((4, N))
    u[0] = 1.0
    for k in range(3):
        s = np.bincount(dst, weights=u[k][src], minlength=N)
        u[k + 1] = recip * s
    pl.u_sl = np.zeros((NCORES, 4, NLR), np.float32)
    for c in range(NCORES):
        m = nodeat[c] >= 0
        pl.u_sl[c][:, m] = u[:, nodeat[c, m]]

    # fold matrices: F_h sums the 4 window-partials of dst-half h
    foldF = np.zeros((128, 32), np.float32)
    for w in range(NW):
        for h in range(2):
            k = 2 * w + h
            foldF[16 * k + np.arange(16), 16 * h + np.arange(16)] = 1.0
    pl.foldF = foldF.astype(BF16)
    return pl


def make_table(pl, yplanes):
    """yplanes: [NCORES, 2, 16, NLR] bf16 (uint16 view ok) -> tab
    [128, NE*2] bf16: partition block 16k = window k//2 (2 NCs, both
    dst-half cores hold the same window). Byte permutation only."""
    tab = np.zeros((128, pl.NE, 2), np.uint16)
    yv = yplanes.view(np.uint16)
    for k in range(NCORES):
        w = k // 2
        for half in range(2):
            c = 2 * w + half
            sl = slice(half * pl.NLR, (half + 1) * pl.NLR)
            tab[16 * k:16 * k + 16, sl, 0] = yv[c, 0]
            tab[16 * k:16 * k + 16, sl, 1] = yv[c, 1]
    return tab.reshape(128, pl.NE * 2).view(BF16)


# ----------------------------------------------------------- device program
def _emit_agg(nc, tc, pl, pools, tab_t, idxg_t, idxr_t, recip_t, foldF_t,
              y_dram0, y_dram1):
    gp, rp, yp, psp, ap_ = pools
    f32 = mybir.dt.float32
    bf = mybir.dt.bfloat16

    ct = ap_.tile([128, (pl.L_T + 1) * 2], bf)
    ctv = ct[:].rearrange("p (e d) -> p e d", d=2)
    nc.vector.memset(ctv[:, pl.L_T, :], 0.0)
    tabv = tab_t[:].rearrange("p (e d) -> p e d", d=2)
    for ci, ch in enumerate(pl.chunks):
        fill = pl.chunk_fill[ci]
        got = gp.tile([128, CH_G * 2], bf, tag="got")
        gv = got[:].rearrange("p (i d) -> p i d", d=2)
        nc.gpsimd.ap_gather(
            out_ap=gv[:, :fill, :], in_ap=tabv,
            idxs_ap=idxg_t[:, ci * CH_G // 16:ci * CH_G // 16 + fill // 16],
            channels=128, num_elems=pl.NE, d=2, num_idxs=fill)
        for (K, r0, nr, soff) in ch:
            pv = gv[:, soff:soff + nr * K, :].rearrange(
                "p (n k) d -> p n k d", k=K)
            kk = K
            while kk > 1:
                h = kk // 2
                nc.vector.tensor_add(out=pv[:, :, :h, :], in0=pv[:, :, :h, :],
                                     in1=pv[:, :, h:2 * h, :])
                if kk % 2 == 1:
                    nc.vector.tensor_add(out=pv[:, :, 0, :],
                                         in0=pv[:, :, 0, :],
                                         in1=pv[:, :, kk - 1, :])
                kk = h
            nc.vector.tensor_copy(out=ctv[:, r0:r0 + nr, :],
                                  in_=pv[:, :, 0, :])
    # reorder to slot order (each core reorders its dst-half) + per-half
    # fold across the 4 windows + scale by recip
    for rc in range((pl.HGP + CH_R - 1) // CH_R):
        ncols = min(CH_R, pl.HGP - rc * CH_R)
        rot = rp.tile([128, CH_R * 2], bf, tag="rot")
        rv = rot[:].rearrange("p (i d) -> p i d", d=2)
        nc.gpsimd.ap_gather(
            out_ap=rv[:, :ncols, :], in_ap=ctv,
            idxs_ap=idxr_t[:, rc * CH_R // 16:rc * CH_R // 16 + ncols // 16],
            channels=128, num_elems=pl.L_T + 1, d=2, num_idxs=ncols)
        ysts = {(h, j): yp.tile([16, CH_R], bf, name=f"yst{h}{j}",
                                tag=f"y{h}{j}")
                for h in range(2) for j in range(2)}
        for f0 in range(0, ncols, FOLD):
            fw = min(FOLD, ncols - f0)
            for h in range(2):
                ps = psp.tile([16, FOLD * 2], f32, space="PSUM", tag="ps")
                nc.tensor.matmul(out=ps[:, :fw * 2],
                                 lhsT=foldF_t[:, 16 * h:16 * h + 16],
                                 rhs=rot[:, f0 * 2:(f0 + fw) * 2],
                                 start=True, stop=True)
                psv = ps[:].rearrange("p (n d) -> p n d", d=2)
                cols = slice(h * pl.HGP + rc * CH_R + f0,
                             h * pl.HGP + rc * CH_R + f0 + fw)
                nc.vector.tensor_mul(out=ysts[(h, 0)][:, f0:f0 + fw],
                                     in0=psv[:, :fw, 0], in1=recip_t[:, cols])
                nc.vector.tensor_mul(out=ysts[(h, 1)][:, f0:f0 + fw],
                                     in0=psv[:, :fw, 1], in1=recip_t[:, cols])
        for h in range(2):
            base = h * pl.HGP + rc * CH_R
            nc.sync.dma_start(out=y_dram0[:, base:base + ncols],
                              in_=ysts[(h, 0)][:, :ncols])
            nc.sync.dma_start(out=y_dram1[:, base:base + ncols],
                              in_=ysts[(h, 1)][:, :ncols])


def prog(pl, tail):
    key = ('prog', tail, pl.NE, pl.L_T, pl.SG, pl.NLR, tuple(pl.chunk_fill))
    if key in _prog_cache:
        return _prog_cache[key]
    nc = bacc.Bacc("TRN2", target_bir_lowering=False, debug=False,
                   num_devices=NCORES)
    f32 = mybir.dt.float32
    f16 = mybir.dt.float16
    bf = mybir.dt.bfloat16
    i16 = mybir.dt.int16

    tab_d = nc.dram_tensor("tab", (128, pl.NE * 2), bf,
                           kind="ExternalInput").ap()
    idxg_d = nc.dram_tensor("idxg", (128, pl.SG // 16), i16,
                            kind="ExternalInput").ap()
    idxr_d = nc.dram_tensor("idxr", (128, pl.HGP // 16), i16,
                            kind="ExternalInput").ap()
    recip_d = nc.dram_tensor("recip", (16, pl.NLR), f16,
                             kind="ExternalInput").ap()
    foldF_d = nc.dram_tensor("foldF", (128, 32), bf,
                             kind="ExternalInput").ap()
    if not tail:
        y0_d = nc.dram_tensor("yj0", (16, pl.NLR), bf,
                              kind="ExternalOutput").ap()
        y1_d = nc.dram_tensor("yj1", (16, pl.NLR), bf,
                              kind="ExternalOutput").ap()
    else:
        y0_d = nc.dram_tensor("y3t0", (16, pl.NLR), bf, kind="Internal").ap()
        y1_d = nc.dram_tensor("y3t1", (16, pl.NLR), bf, kind="Internal").ap()
        yin0_d = nc.dram_tensor("yin0", (48, pl.NLR), bf,
                                kind="ExternalInput").ap()
        yin1_d = nc.dram_tensor("yin1", (48, pl.NLR), bf,
                                kind="ExternalInput").ap()
        u_d = nc.dram_tensor("u", (4, pl.NLR), bf, kind="ExternalInput").ap()
        mask_d = nc.dram_tensor("maskv", (1, pl.NLR), bf,
                                kind="ExternalInput").ap()
        ct_d = nc.dram_tensor("CT", (128, 128), bf, kind="ExternalInput").ap()
        wo_d = nc.dram_tensor("Wo", (64, 8), f32, kind="ExternalInput").ap()
        bo_d = nc.dram_tensor("bo", (8, 8), f32, kind="ExternalInput").ap()
        out_d = nc.dram_tensor("out", (8, 8), f32, kind="ExternalOutput").ap()

    with tile.TileContext(nc) as tc:
        with tc.tile_pool(name="persist", bufs=1) as pp:
            idxg_t = pp.tile([128, pl.SG // 16], i16)
            nc.sync.dma_start(out=idxg_t[:], in_=idxg_d[:, :])
            idxr_t = pp.tile([128, pl.HGP // 16], i16)
            nc.sync.dma_start(out=idxr_t[:], in_=idxr_d[:, :])
            recip_t = pp.tile([16, pl.NLR], f16)
            nc.sync.dma_start(out=recip_t[:], in_=recip_d[:, :])
            foldF_t = pp.tile([128, 32], bf)
            nc.sync.dma_start(out=foldF_t[:], in_=foldF_d[:, :])

            with tc.tile_pool(name="agg", bufs=1) as ap_, \
                 tc.tile_pool(name="g", bufs=2) as gp, \
                 tc.tile_pool(name="r", bufs=2) as rp, \
                 tc.tile_pool(name="ys", bufs=1) as yp, \
                 tc.tile_pool(name="ps", bufs=4, space="PSUM") as psp:
                tab_t = ap_.tile([128, pl.NE * 2], bf)
                nc.sync.dma_start(out=tab_t[:], in_=tab_d[:, :])
                _emit_agg(nc, tc, pl, (gp, rp, yp, psp, ap_), tab_t, idxg_t,
                          idxr_t, recip_t, foldF_t, y0_d, y1_d)

            if tail:
                with tc.tile_pool(name="tail", bufs=1) as tp, \
                     tc.tile_pool(name="ps2", bufs=4, space="PSUM") as psp2:
                    Y0 = tp.tile([128, pl.NLR], bf)
                    Y1 = tp.tile([128, pl.NLR], bf)
                    nc.vector.memset(Y0[:], 0.0)
                    nc.vector.memset(Y1[:], 0.0)
                    # y3 at rows 0-15 (round trip via internal dram)
                    nc.sync.dma_start(out=Y0[0:16, :], in_=y0_d[:, :])
                    nc.sync.dma_start(out=Y1[0:16, :], in_=y1_d[:, :])
                    # y0,y1,y2 at rows 32/64/96
                    for i in range(3):
                        nc.sync.dma_start(out=Y0[32 * (i + 1):32 * (i + 1) + 16, :],
                                          in_=yin0_d[16 * i:16 * i + 16, :])
                        nc.sync.dma_start(out=Y1[32 * (i + 1):32 * (i + 1) + 16, :],
                                          in_=yin1_d[16 * i:16 * i + 16, :])
                    # u rows: layer k block row +16 (block order y3,y0,y1,y2)
                    for blk, k in ((0, 3), (1, 0), (2, 1), (3, 2)):
                        nc.sync.dma_start(out=Y0[32 * blk + 16:32 * blk + 17, :],
                                          in_=u_d[k:k + 1, :])
                        nc.sync.dma_start(out=Y1[32 * blk + 16:32 * blk + 17, :],
                                          in_=u_d[k:k + 1, :])
                    # mask row 17 (plane 0 only)
                    nc.sync.dma_start(out=Y0[17:18, :], in_=mask_d[:, :])
                    CT_t = tp.tile([128, 128], bf)
                    nc.sync.dma_start(out=CT_t[:], in_=ct_d[:, :])
                    h3sb = tp.tile([64, pl.NLR], bf)
                    for f0 in range(0, pl.NLR, FOLD):
                        fw = min(FOLD, pl.NLR - f0)
                        hp = psp2.tile([64, FOLD], f32, space="PSUM", tag="hp")
                        nc.tensor.matmul(out=hp[:, :fw], lhsT=CT_t[:, 0:64],
                                         rhs=Y0[:, f0:f0 + fw],
                                         start=True, stop=False)
                        nc.tensor.matmul(out=hp[:, :fw], lhsT=CT_t[:, 64:128],
                                         rhs=Y1[:, f0:f0 + fw],
                                         start=False, stop=True)
                        nc.scalar.copy(out=h3sb[:, f0:f0 + fw], in_=hp[:, :fw])
                    pooled = tp.tile([64, 8], f32)
                    for g in range(8):
                        nc.vector.tensor_reduce(
                            out=pooled[:, g:g + 1],
                            in_=h3sb[:, g * pl.GP:(g + 1) * pl.GP],
                            axis=mybir.AxisListType.X, op=mybir.AluOpType.max)
                    wo_t = tp.tile([64, 8], f32)
                    nc.sync.dma_start(out=wo_t[:], in_=wo_d[:, :])
                    bo_t = tp.tile([8, 8], f32)
                    nc.sync.dma_start(out=bo_t[:], in_=bo_d[:, :])
                    lg = psp2.tile([8, 8], f32, space="PSUM", tag="lg")
                    nc.tensor.matmul(out=lg[:], lhsT=pooled[:], rhs=wo_t[:],
                                     start=True, stop=True)
                    lgs = tp.tile([8, 8], f32)
                    nc.vector.tensor_add(out=lgs[:], in0=lg[:], in1=bo_t[:])
                    mx = tp.tile([8, 1], f32)
                    nc.vector.tensor_reduce(out=mx[:], in_=lgs[:],
                                            axis=mybir.AxisListType.X,
                                            op=mybir.AluOpType.max)
                    nc.vector.tensor_scalar(out=lgs[:], in0=lgs[:],
                                            scalar1=mx[:, :1], scalar2=None,
                                            op0=mybir.AluOpType.subtract)
                    ex = tp.tile([8, 8], f32)
                    nc.scalar.activation(out=ex[:], in_=lgs[:],
                                         func=mybir.ActivationFunctionType.Exp)
                    sm = tp.tile([8, 1], f32)
                    nc.vector.tensor_reduce(out=sm[:], in_=ex[:],
                                            axis=mybir.AxisListType.X,
                                            op=mybir.AluOpType.add)
                    lns = tp.tile([8, 1], f32)
                    nc.scalar.activation(out=lns[:], in_=sm[:],
                                         func=mybir.ActivationFunctionType.Ln)
                    nc.vector.tensor_scalar(out=lgs[:], in0=lgs[:],
                                            scalar1=lns[:, :1], scalar2=None,
                                            op0=mybir.AluOpType.subtract)
                    nc.sync.dma_start(out=out_d[:, :], in_=lgs[:])
    nc.compile()
    _prog_cache[key] = nc
    return nc


# ----------------------------------------------------------------- execution
def _run(nc, in_maps, outputs):
    if SIM:
        import concourse.bass_interp as bi
        sim = bi.MultiCoreSim(nc, num_cores=NCORES)
        for cid, cs in sim.cores.items():
            for k, v in in_maps[cid].items():
                cs.tensor(k)[:] = v
        sim.simulate()
        return [{o: np.array(sim.cores[c].tensor(o)) for o in outputs}
                for c in range(NCORES)]
    res = bass_utils.run_bass_kernel_spmd(nc, in_maps,
                                          core_ids=list(range(NCORES)),
                                          trace=TRACE)
    if TRACE:
        LAST_EXEC_NS.append(res.exec_time_ns)
    return res.results


def kernel(**inputs):
    x = np.asarray(inputs['x'], np.float32)
    edge_index = np.asarray(inputs['edge_index'])
    batch = np.asarray(inputs['batch'])
    N = x.shape[0]
    G = int(batch.max()) + 1
    F_IN = x.shape[1]
    pl = build_plan(edge_index, batch, N, G)

    # folded coefficients (host, weights only — same as prior kernel)
    Wl = [np.asarray(inputs[f'Wl{i}'], np.float64) for i in range(3)]
    Wr = [np.asarray(inputs[f'Wr{i}'], np.float64) for i in range(3)]
    bl = [np.asarray(inputs[f'bl{i}'], np.float64) for i in range(3)]
    C0 = Wr[0] @ Wr[1] @ Wr[2]
    C1 = Wr[0] @ Wr[1] @ Wl[2] + Wr[0] @ Wl[1] @ Wr[2] + Wl[0] @ Wr[1] @ Wr[2]
    C2 = Wr[0] @ Wl[1] @ Wl[2] + Wl[0] @ Wr[1] @ Wl[2] + Wl[0] @ Wl[1] @ Wr[2]
    C3 = Wl[0] @ Wl[1] @ Wl[2]
    d0 = bl[0] @ Wr[1] @ Wr[2] + bl[1] @ Wr[2] + bl[2]
    d1 = bl[0] @ (Wr[1] @ Wl[2] + Wl[1] @ Wr[2]) + bl[1] @ Wl[2]
    d2 = bl[0] @ Wl[1] @ Wl[2]
    d3 = np.zeros(64)
    Cs = {0: (C0, d0), 1: (C1, d1), 2: (C2, d2), 3: (C3, d3)}
    # CT[j][128, 64]: partition block 32*blk (blk order: y3,y0,y1,y2)
    CT = np.zeros((2, 128, 64), np.float32)
    for blk, k in ((0, 3), (1, 0), (2, 1), (3, 2)):
        Cm, dv = Cs[k]
        for j in range(2):
            rows = 2 * np.arange(16) + j
            CT[j, 32 * blk + np.arange(16)] = Cm[rows]
            CT[j, 32 * blk + 16] = dv / 2.0
    CT[0, 17] = 1.0  # mask row (plane 0)
    CT_in = np.concatenate([CT[0], CT[1]], axis=1).astype(BF16)  # [128,128]

    # initial table from x (bf16, slot order, feature-pair layout)
    xb = x.astype(BF16).view(np.uint16)
    y0planes = np.zeros((NCORES, 2, 16, pl.NLR), np.uint16)
    for c in range(NCORES):
        m = pl.nodeat[c] >= 0
        nodes = pl.nodeat[c, m]
        for j in range(2):
            # plane j, partition p = feature 2p+j
            y0planes[c, j][:, m] = xb[nodes][:, 2 * np.arange(16) + j].T
    tab0 = make_table(pl, y0planes.view(BF16))

    recip16 = pl.recip_sl.astype(np.float16)
    statics = []
    for c in range(NCORES):
        statics.append({
            "idxg": pl.idxg_w[c],
            "idxr": pl.idxr_w[c],
            "recip": np.broadcast_to(recip16[c][None, :], (16, pl.NLR)).copy(),
            "foldF": pl.foldF,
        })

    nc_agg = prog(pl, tail=False)
    yplanes = [y0planes]
    tab = tab0
    for layer in range(2):
        in_maps = [dict(statics[c], tab=tab) for c in range(NCORES)]
        res = _run(nc_agg, in_maps, ["yj0", "yj1"])
        yp = np.zeros((NCORES, 2, 16, pl.NLR), np.uint16)
        for c in range(NCORES):
            yp[c, 0] = np.asarray(res[c]["yj0"]).view(np.uint16)
            yp[c, 1] = np.asarray(res[c]["yj1"]).view(np.uint16)
        yplanes.append(yp)
        tab = make_table(pl, yp.view(BF16))

    nc_tail = prog(pl, tail=True)
    in_maps = []
    for c in range(NCORES):
        m = dict(statics[c], tab=tab)
        yin0 = np.concatenate([yplanes[k][c, 0] for k in range(3)], axis=0)
        yin1 = np.concatenate([yplanes[k][c, 1] for k in range(3)], axis=0)
        m["yin0"] = yin0.view(BF16)
        m["yin1"] = yin1.view(BF16)
        m["u"] = pl.u_sl[c].astype(BF16)
        m["maskv"] = pl.mask_sl[c][None, :].astype(BF16)
        m["CT"] = CT_in
        m["Wo"] = np.asarray(inputs['W_out'], np.float32)
        m["bo"] = np.asarray(inputs['b_out'], np.float32)[None, :].repeat(8, 0)
        in_maps.append(m)
    res = _run(nc_tail, in_maps, ["out"])

    out = np.zeros((G, 8), np.float32)
    for c in range(NCORES):
        out[c * pl.gpc:(c + 1) * pl.gpc] = res[c]["out"]
    return out
